# revision 76
# baseline (speedup 1.0000x reference)
"""Trainium2 Bass kernel for the MoE-Adapter module.

Math (per token):
  y = x @ W_base + b_base                       (dense base, stop-grad)
    + (x @ A_s) @ B_s                           (shared rank-16 LoRA)
    + sum_e w_e * (x @ A_r[e]) @ B_r[e]         (6 routed rank-16 LoRA experts)
  w = softmax(x @ W_router + b_router) masked to its top-2 entries

Strategy: data-parallel over the 16384 tokens across 8 NeuronCores (2048
tokens/core); all weights replicated.  Compute in fp16 (full PE rate,
fp32 PSUM accumulation).  The router is computed in fp16-pair precision
(x = xh + xl, W_router = Wrh + Wrl, logits = xh@Wrh + xh@Wrl + xl@Wrh)
so top-2 selection matches the fp32 reference.

Per-core layout: the host stages x as an fp16 pair in transposed,
token-tile-major layout (xhT/xlT: [tile, din%128, ktile*128+tok]) and
W_base as fp16, so every device-side load is one contiguous DMA (the
HWDGE direct2d descriptor only has room for 2 sync-wait commands, which
rules out multi-dependency staging chains on-chip).  The rank-space
projections of all 7 experts plus both router column groups are packed
into one 128-wide rhs so the whole adapter+router down-projection is a
single accumulation chain.  Gate weights are applied in rank space, the
scaled rank vectors are transposed on the PE and folded into the base
matmul's PSUM accumulation together with the bias (K=1 ones matmul).

The last NK8=4 k-tiles of the base matmul run in fp8 e4m3 with
perf_mode=DoubleRow (2 fp8 MACs/PE-cell/cycle): x is quantized host-side
straight from fp32, W's tail is quantized at 64x so its values sit in
e4m3's normal range, and the entire Y accumulation runs at 64x (W16, B_r,
B_s, b_base are host-scaled) so fp16 and fp8 partial products share one
PSUM group; the PSUM->SBUF copy applies the 1/64.  Four k-tiles is the
most the 2e-2 abs-max error budget allows (measured rel err 0.0178, vs
0.0003 all-fp16; the error is ~100% fp8 quantization, split evenly
between the x and W sides, so NK8=6 would land at 0.0218).

Measured refinements over the original baseline (297.7us -> ~285us on
the same measurement setup):
 - fp8 DoubleRow matmuls stream the full 512-col psum bank (FP8W): the
   512-cycle stream covers the next matmul's ~213ns no-FWL LDWEIGHTS via
   the background-weight-buffer ping-pong.  Ablation: the fp8 tail now
   costs ~135ns/MM vs ~216ns for an equivalent fp16 MM (fp8 saves 37us
   vs an all-fp16 tail).
 - y leaves the device as fp16 (Y16) and xl as e4m3 at 2^15 scale (XL8,
   own 2^26-scaled psum columns; start=False always since a start=True
   matmul clears the WHOLE psum bank's has_written bits - measured).
 - the gating-independent base matmuls are emitted before the gating
   chain so the serial DVE/ACT chain overlaps them (s5_open/s5_close).
 - DMA is fully hidden (noxdma+noydma ablation: no change); stage-1
   costs its exact PE streaming time (20us of 290); the kernel is
   PE-streaming-bound, so fp16 1-col/cycle sets the floor.
Dead ends (measured): dropping the xl router correction flips top-2 for
1/16384 tokens and lands rel err at 0.0231 (over budget); computing
stage 1 transposed (TR1) to kill the PE transpose measured +1.4us.
"""

import os
import sys

import numpy as np

for _p in ("/opt/trn_rl_repo",):
    if os.path.isdir(_p) and _p not in sys.path:
        sys.path.insert(0, _p)

import concourse.bass as bass
import concourse.mybir as mybir
import concourse.tile as tile
from concourse import bacc
from concourse import bass_utils
from concourse.masks import make_identity

B, S, D, E, R = 4, 4096, 2048, 6, 16
NCORES = 8
NTOK = B * S               # 16384 tokens total
P = 128
KT = D // P                # 16 k-tiles over the contraction dim
NK8 = 4                    # trailing k-tiles done in fp8 e4m3 DoubleRow
KT16 = KT - NK8            # leading k-tiles done in fp16
D16 = KT16 * P             # contraction split point
NCHUNK = 512               # PSUM bank width (fp32)
NCH = D // NCHUNK          # 4 output column chunks
NEG = -60000.0             # exp() flushes this to 0; fits in fp16

F32 = mybir.dt.float32
F16 = mybir.dt.float16
F8 = mybir.dt.float8e4
SWI = False                # DoubleRowSwInterleave: measured equal to DoubleRow
ILV = False                # interleaving fp8 between fp16 k-tiles measured
                           # +11us/rep WORSE than the block order (breaks the
                           # fp16 FWL prefetch chain); keep the fp8 block
CONS = True                # one fp8 block per tile (fewer FWL<->DoubleRow
                           # mode switches); measured ~-6us/rep vs split
XLI = True                 # interleave xl router matmuls between xh k-tiles:
                           # hides their LDWEIGHTS, measured -7us/rep
ABLATE = frozenset()       # timing-probe ablations (break correctness):
                           # "noxdma" (x DMAs only for t==0), "noydma",
                           # "nostage1" (skip adapter/router/gating),
                           # "nofp8" (tail k-tiles as fp16 on reused planes)
HSPL = True                # pipeline the two 1024-col halves: close+copy of
                           # h=0 overlaps h=1's base matmuls, freeing PSUM
                           # banks mid-tile (kills the per-tile bank stall)
XL8 = True                 # xl residual travels as e4m3 (halves xl DMA)
XL0 = False                # drop the xl router correction: MEASURED OVER
                           # BUDGET (rel 0.0231) -- the 1/16384 top-2 flip
                           # lands where the fp8 error is already large
FP8W = True                # fp8 base matmuls stream 512 cols (hides LDW)
Y16 = True                 # y leaves the device as fp16 (halves out DMA)
TR1 = False                # stage 1 computes H^T directly (stationary=AR,
                           # moving=xhT), killing the PE transpose; gating
                           # runs off small DVE 32x32 block transposes and
                           # the gate scales fold in via a K=8 expand matmul


def _stage5_parts(nc, tc, psY, yout, Wk, W8sb, x8sb, Bc, y_d, t, xhT,
                  nchunk, nch):
    """Base matmul + adapter up-projection (incl. bias row), fused in PSUM.

    Returns (emit_open, emit_close_all): open emits the gating-INDEPENDENT
    fp16+fp8 base matmuls; close_all(HsT) emits the adapter close matmuls,
    PSUM->SBUF copies and the y DMA.  Splitting them lets the ~1.5-2.5us
    serial DVE/ACT gating chain run under the ~11us of base matmuls instead
    of stalling the PE at the first gating-dependent instruction (the PE
    executes its queue in emission order).

    The whole Y accumulation runs at 64x natural scale (W16/W8/Bc/bias are
    host-scaled by 64) so the fp8 tail k-tiles land in e4m3's normal range;
    the PSUM->SBUF copy applies the 1/64."""
    ysb = yout.tile([P, D], F16 if Y16 else F32, tag="ysb", name=f"ysb_{t}")
    pm = (mybir.MatmulPerfMode.DoubleRowSwInterleave if SWI
          else mybir.MatmulPerfMode.DoubleRow)
    hs = list(range(nch // 2))
    psys = {h: [psY.tile([P, nchunk], F32, tag=f"psy{n}",
                         name=f"psy{h}_{n}_{t}") for n in range(2)]
            for h in hs}
    # The NK8 trailing k-tiles run in fp8 e4m3 DoubleRow: each instr loads
    # two full 128x128 stationary planes (one per k-tile of the pair) and
    # streams the full 512-col psum bank of the paired W8 rows.  512-wide
    # streams (241ns) cover the 213ns no-FWL LDWEIGHTS of the NEXT matmul
    # (background weight buffer ping-pong), so consecutive jobs alternate
    # the k-pair.  The fp8 matmuls stay in back-to-back blocks: interleaving
    # them with fp16 matmuls measured +11us/rep worse (breaks the fp16 FWL
    # prefetch chain).
    if FP8W:
        fp8_jobs = [(n, kp, 0) for n in range(2) for kp in range(NK8 // 2)]
    else:
        fp8_jobs = [(j % 2, (j // 2) % 2, j // 4) for j in range(2 * NK8)]
    fp8_w = nchunk if FP8W else 256

    def emit_fp16(h):
        for kt in range(KT16):
            for n in range(2):
                lo = (2 * h + n) * nchunk
                nc.tensor.matmul(psys[h][n][:], xhT[:, kt * P:(kt + 1) * P],
                                 Wk[kt][:, lo:lo + nchunk],
                                 start=(kt == 0), stop=False)

    def emit_fp8(h, job):
        n, kp, ch = job
        lo = (2 * h + n) * nchunk
        if "nofp8" in ABLATE:
            for kk in range(2):
                kt = 2 * kp + kk
                nc.tensor.matmul(psys[h][n][:],
                                 xhT[:, (KT16 + kt) * P:(KT16 + kt + 1) * P],
                                 Wk[kt][:, lo:lo + nchunk],
                                 start=False, stop=False,
                                 skip_group_check=True)
            return
        c0 = fp8_w * ch
        nc.tensor.matmul(
            psys[h][n][:, c0:c0 + fp8_w],
            x8sb[:, kp, :, :],
            W8sb[kp][:, :, lo + c0:lo + c0 + fp8_w],
            start=False, stop=False,
            perf_mode=pm,
            skip_group_check=True)

    def emit_close(h, HsT):
        for n in range(2):
            lo = (2 * h + n) * nchunk
            nc.tensor.matmul(psys[h][n][:], HsT[0:113],
                             Bc[0:113, lo:lo + nchunk],
                             start=False, stop=True)
            nc.scalar.activation(ysb[:, lo:lo + nchunk], psys[h][n][:],
                                 mybir.ActivationFunctionType.Copy,
                                 scale=1.0 / 64.0)

    def _ydma():
        if "noydma" in ABLATE:
            nc.sync.dma_start(y_d[t * P:t * P + 1, :], ysb[0:1, :])
        else:
            nc.sync.dma_start(y_d[t * P:(t + 1) * P, :], ysb[:])

    if HSPL:
        # column-half pipelining: h=0's base+fp8 in the open phase; its
        # close + copies drain that half's PSUM banks while h=1's base
        # matmuls run, so the NEXT tile's h=0 matmuls never wait on a bank
        # (the bank-recycle stall also re-throttled the PE p-state)
        def emit_open():
            emit_fp16(hs[0])
            for job in fp8_jobs:
                emit_fp8(hs[0], job)

        def emit_close_all(HsT):
            emit_close(hs[0], HsT)
            for h in hs[1:]:
                emit_fp16(h)
                for job in fp8_jobs:
                    emit_fp8(h, job)
                emit_close(h, HsT)
            _ydma()
    else:
        def emit_open():
            # one fp8 block per tile: fewer FWL<->DoubleRow mode switches
            for h in hs:
                emit_fp16(h)
            for h in hs:
                for job in fp8_jobs:
                    emit_fp8(h, job)

        def emit_close_all(HsT):
            for h in hs:
                emit_close(h, HsT)
            _ydma()

    return emit_open, emit_close_all


def build_kernel(T: int, repeat: int = 1, nchunk: int = NCHUNK,
                 depths: tuple = (6, 4, 4)) -> bacc.Bacc:
    """Build the per-core kernel for T tokens (T % 128 == 0).

    repeat > 1 wraps the main loop in a device-side For_i that redoes the
    whole computation; used only for wall-clock timing (amplifies kernel
    time far above the dispatch noise)."""
    assert XL8 or not TR1, "TR1 needs the XL8 xl layout (separate psum cols)"
    TT = T // P
    nch = D // nchunk
    xt_b, gate_b, yout_b = depths
    nc = bacc.Bacc("TRN2", target_bir_lowering=False, debug=False)

    xhT_d = nc.dram_tensor("xhT", [T // P, P, D], F16, kind="ExternalInput").ap()
    # xl (the fp16 residual of x, used only for the router's low-order
    # logit correction) travels as e4m3 at 2^15 scale: |xl| <= 2.5e-3 so
    # 2^15*xl <= ~83 sits in e4m3's normal range; its 2^-4 relative quant
    # error perturbs logits by ~5e-6, far below the ~1e-4 logit gaps that
    # matter for top-2 selection.  Halves the xl DMA traffic vs fp16.
    xlT_d = (None if XL0 else
             nc.dram_tensor("xl8T" if XL8 else "xlT", [T // P, P, D],
                            F8 if XL8 else F16, kind="ExternalInput").ap())
    x8_d = (nc.dram_tensor("x8T", [T // P, P, NK8 // 2, 2, P], F8,
                           kind="ExternalInput").ap() if NK8 else None)
    Wb_d = nc.dram_tensor("W16", [D16, D], F16, kind="ExternalInput").ap()
    W8_d = (nc.dram_tensor("W8", [NK8 // 2, P, 2, D], F8,
                           kind="ExternalInput").ap() if NK8 else None)
    bb_d = nc.dram_tensor("b_base", [1, D], F32, kind="ExternalInput").ap()
    As_d = nc.dram_tensor("A_s", [D, R], F32, kind="ExternalInput").ap()
    Bs_d = nc.dram_tensor("B_s", [R, D], F32, kind="ExternalInput").ap()
    Ar_d = nc.dram_tensor("A_r", [E, D, R], F32, kind="ExternalInput").ap()
    Br_d = nc.dram_tensor("B_r", [E, R, D], F32, kind="ExternalInput").ap()
    Wr_d = nc.dram_tensor("W_router", [D, E], F32, kind="ExternalInput").ap()
    br_d = nc.dram_tensor("b_router", [1, E], F32, kind="ExternalInput").ap()
    expt_d = ones_d = None
    if TR1:
        expt_d = nc.dram_tensor("EXPT", [9, 112], F16, kind="ExternalInput").ap()
        ones_d = nc.dram_tensor("onesrow", [32, P], F16, kind="ExternalInput").ap()
    # y leaves the device as fp16 (values <= ~6, fp16 rounding adds <= 4e-4
    # relative error); the host upcasts.  Halves the output DMA traffic.
    y_d = nc.dram_tensor("y", [T, D], F16, kind="ExternalOutput").ap()

    with tile.TileContext(nc) as tc:
        with (
            tc.tile_pool(name="const", bufs=1) as const,
            tc.tile_pool(name="wpool", bufs=1) as wpool,
            tc.tile_pool(name="small", bufs=1) as small,
            tc.tile_pool(name="xT", bufs=xt_b) as xTp,
            tc.tile_pool(name="gate", bufs=gate_b) as gate,
            tc.tile_pool(name="yout", bufs=yout_b) as yout,
            tc.tile_pool(name="psY", bufs=(2 if nchunk <= 512 else 1),
                         space="PSUM") as psY,
            tc.tile_pool(name="psH", bufs=2, space="PSUM") as psH,
            tc.tile_pool(name="psT", bufs=2, space="PSUM") as psT,
        ):
            # ---- constants ----
            ones = const.tile([1, P], F16)
            nc.vector.memset(ones[:], 1.0)
            ident = const.tile([P, P], F16)
            make_identity(nc, ident[:])

            # brow: K=1 bias row for the adapter/router chain
            # [0]*112 | b_router (6) | NEG pads (2) | 0 (8)
            brow = const.tile([1, P], F16)
            nc.vector.memset(brow[:], 0.0)
            brs = small.tile([1, E], F32, tag="brs")
            nc.sync.dma_start(brs[:], br_d[:])
            nc.vector.tensor_copy(brow[:, 112:118], brs[:])
            nc.vector.memset(brow[:, 118:120], NEG)

            # broadcast the router-bias row to all 128 partitions once (K=1
            # ones matmul); the per-tile Lsb extraction adds it on the DVE,
            # replacing a per-tile 128-col PE matmul
            psh = psH.tile([P, P], F32)
            nc.tensor.matmul(psh[:, 0:8], ones[:], brow[:, 112:120],
                             start=True, stop=True)
            brB = const.tile([P, 8], F32)
            nc.vector.tensor_copy(brB[:], psh[:, 0:8])

            # b_base is folded into the adapter up-projection as rank row
            # 112 (its HsT row is a constant 1.0 column in rank space)

            # EXPT (DMA'd host const — compute engines can't write at
            # unaligned partition bases): EXPT[e, 16e:16e+16] = 1 for the 6
            # experts, EXPT[8, 96:112] = 1 so the shared rows scale by the
            # constant-1 row of wgtT.  One K=9 matmul expands wgtT [9, tok]
            # to per-rank-row scales S^T [112, tok].
            EXPT = None
            if TR1:
                EXPT = const.tile([9, 112], F16)
                nc.sync.dma_start(EXPT[:], expt_d[:])
                # MASK/ONESROW for HsT rows 96:128: shared rows pass through,
                # row 112 becomes the constant bias row, 113:128 zero.  Both
                # live at partitions 96:128 of full-height tiles (engine ops
                # need equal base partitions across SBUF operands; ONESROW
                # is DMA'd since compute can't write at partition 112).
                MASK = const.tile([128, P], F16)
                nc.vector.memset(MASK[96:128, :], 0.0)
                nc.vector.memset(MASK[96:112, :], 1.0)
                ONESROW = const.tile([128, P], F16)
                nc.sync.dma_start(ONESROW[96:128, :], ones_d[:])

            # ---- adapter down-proj + router rhs: [P, KT, 128] fp16 ----
            # cols: A_r[e]*6 (96) | A_s(16) | Wrh(6) | 0(2) | Wrl(6) | 0(2)
            AR = const.tile([P, KT, P], F16)
            nc.vector.memset(AR[:, :, 118:120], 0.0)
            nc.vector.memset(AR[:, :, 126:128], 0.0)
            for e in range(E):
                art = small.tile([P, KT, R], F32, tag=f"art{e}")
                nc.sync.dma_start(art[:], Ar_d[e].rearrange("(k p) r -> p k r", p=P))
                nc.vector.tensor_copy(AR[:, :, 16 * e:16 * (e + 1)], art[:])
            ast = small.tile([P, KT, R], F32, tag="ast")
            nc.sync.dma_start(ast[:], As_d.rearrange("(k p) r -> p k r", p=P))
            nc.vector.tensor_copy(AR[:, :, 96:112], ast[:])
            wrt = small.tile([P, KT, E], F32, tag="wrt")
            nc.sync.dma_start(wrt[:], Wr_d.rearrange("(k p) e -> p k e", p=P))
            nc.vector.tensor_copy(AR[:, :, 112:118], wrt[:])
            # Wrl = fp32(Wr) - fp16(Wr), rounded to fp16
            nc.vector.tensor_sub(AR[:, :, 120:126], wrt[:], AR[:, :, 112:118])
            # e4m3 copy of Wrh at 2^11 scale (max |Wrh| ~ 0.09 -> ~190, in
            # range) for the fp8 xl router matmuls; the 1.8% quant error
            # only multiplies the tiny xl residual, so it is negligible.
            ARl8 = None
            if XL8:
                ARl8 = const.tile([P, KT, 8], F8)
                nc.vector.memset(ARl8[:, :, 6:8], 0.0)
                nc.scalar.activation(ARl8[:, :, 0:6], AR[:, :, 112:118],
                                     mybir.ActivationFunctionType.Copy,
                                     scale=2048.0)

            # ---- up-proj weights + bias row: rows 0:96 = B_r, 96:112 =
            # B_s, 112 = b_base (the matching HsT row is constant 1.0).
            # Rows 113:128 are zero padding, never read (K=113); compute-
            # engine partition bases must be 32-aligned, hence the [96:128)
            # staging block.
            Bc = const.tile([128, D], F16)
            bst = small.tile([96, D], F32, tag="bst")
            nc.sync.dma_start(bst[:], Br_d.rearrange("e r d -> (e r) d"))
            nc.scalar.activation(Bc[0:96, :], bst[:],
                                 mybir.ActivationFunctionType.Copy)
            bst2 = small.tile([32, D], F32, tag="bst2")
            nc.vector.memset(bst2[:], 0.0)
            nc.sync.dma_start(bst2[0:16, :], Bs_d[:])
            nc.sync.dma_start(bst2[16:17, :], bb_d[:])
            nc.scalar.activation(Bc[96:128, :], bst2[:],
                                 mybir.ActivationFunctionType.Copy)

            # ---- base weight fp16 k-tiles (host pre-cast to fp16, x64) ----
            Wk = []
            for kt in range(KT16):
                wk = wpool.tile([P, D], F16, tag=f"w{kt}")
                nc.sync.dma_start(wk[:], Wb_d[kt * P:(kt + 1) * P, :])
                Wk.append(wk)
            # trailing k-tiles as fp8 e4m3 (x64), packed [p, half, col],
            # one tile per k-pair (keeps slots the same 4KB as wk tiles)
            W8sb = []
            for kp in range(NK8 // 2):
                w8 = wpool.tile([P, 2, D], F8, tag=f"w8_{kp}")
                nc.sync.dma_start(w8[:], W8_d[kp])
                W8sb.append(w8)

            hst_const = None
            if "nostage1" in ABLATE:
                hst_const = const.tile([113, P], F16)
                nc.vector.memset(hst_const[:], 0.5)

            # ---- main loop over 128-token tiles ----
            import contextlib
            rep_ctx = (tc.For_i(0, repeat, 1) if repeat > 1
                       else contextlib.nullcontext())
            with rep_ctx:
              # tile t's close matmuls are deferred until after tile t+1's
              # stage-1 matmuls so the close's wait on the HsT DVE copy is
              # covered by extra PE work (sim: removes ~0.7us/tile PE gap)
              pending_close = [None]
              for t in range(TT):
                  thin = "noxdma" in ABLATE and t > 0
                  xhT = xTp.tile([P, D], F16, tag="xhT")
                  nc.sync.dma_start(xhT[0:1, :] if thin else xhT[:],
                                    xhT_d[t, 0:1] if thin else xhT_d[t])
                  xlT = None
                  if not XL0:
                      xlT = xTp.tile([P, D], F8 if XL8 else F16, tag="xlT")
                      nc.sync.dma_start(xlT[0:1, :] if thin else xlT[:],
                                        xlT_d[t, 0:1] if thin else xlT_d[t])
                  x8sb = None
                  if NK8:
                      x8sb = xTp.tile([P, NK8 // 2, 2, P], F8, tag="x8T")
                      nc.sync.dma_start(x8sb[0:1] if thin else x8sb[:],
                                        x8_d[t, 0:1] if thin else x8_d[t])

                  # stage 1: rank-space projections + router logits.  The
                  # 6-col xl matmuls (fp8, own 2^26-scaled accumulation in
                  # cols 128:134 of the same psum bank) are interleaved
                  # between the 128-col xh matmuls (XLI) so each xl
                  # LDWEIGHTS hides under an xh stream; back-to-back 6-col
                  # matmuls are load-serialized.
                  if "nostage1" in ABLATE:
                      op, cl = _stage5_parts(nc, tc, psY, yout, Wk, W8sb,
                                             x8sb, Bc, y_d, t, xhT,
                                             nchunk, nch)
                      op()
                      cl(hst_const)
                      continue
                  psh = psH.tile([P, 136], F32)

                  def emit_xl(kt):
                      if XL0:
                          return
                      if XL8:
                          # start=False always: the xh kt0 matmul's
                          # start=True clears the WHOLE bank (measured), so
                          # cols 128:134 begin cleared and the first write
                          # lands fresh; start=True here would wipe the xh
                          # kt0 partial.
                          nc.tensor.matmul(psh[:, 128:134],
                                           xlT[:, kt * P:(kt + 1) * P],
                                           ARl8[:, kt, 0:6], start=False,
                                           stop=(kt == KT - 1),
                                           skip_group_check=True)
                      else:
                          nc.tensor.matmul(psh[:, 112:118],
                                           xlT[:, kt * P:(kt + 1) * P],
                                           AR[:, kt, 112:118], start=False,
                                           stop=(kt == KT - 1),
                                           skip_group_check=True)

                  def emit_xh(kt):
                      if TR1:
                          # H^T [rank, tok]: stationary=AR tile, moving=xhT
                          nc.tensor.matmul(psh[:, 0:128], AR[:, kt, :],
                                           xhT[:, kt * P:(kt + 1) * P],
                                           start=(kt == 0), stop=False)
                      else:
                          nc.tensor.matmul(psh[:, 0:128],
                                           xhT[:, kt * P:(kt + 1) * P],
                                           AR[:, kt, :], start=(kt == 0),
                                           stop=False)

                  if XLI:
                      for kt in range(KT):
                          emit_xh(kt)
                          emit_xl(kt)
                  else:
                      for kt in range(KT):
                          emit_xh(kt)
                      for kt in range(KT):
                          emit_xl(kt)

                  # deferred close of the PREVIOUS tile: its gating/HsT chain
                  # is long done (covered by the previous tile's base matmuls
                  # plus this tile's stage 1)
                  if pending_close[0] is not None:
                      pending_close[0]()
                      pending_close[0] = None

                  # stage 5a: the gating-independent base fp16+fp8 matmuls go
                  # into the PE queue NOW, so the serial gating chain below
                  # (DVE/ACT, ~2us) overlaps them instead of stalling the PE
                  s5_open, s5_close = _stage5_parts(nc, tc, psY, yout, Wk,
                                                    W8sb, x8sb, Bc, y_d, t,
                                                    xhT, nchunk, nch)
                  s5_open()

                  # stage 2: top-2 gating  w = softmax(L) * (L >= secondmax(L))
                  Lsb = gate.tile([P, 8], F32, tag="Lsb")
                  if TR1:
                      # H^T rows 96:128 hold [shared(16) | Wrh logits(6) |
                      # 0(2) | Wrl logits(6) | 0(2)]^T; DVE 32x32 block
                      # transposes bring the router rows into token-major
                      Lt32 = gate.tile([P, 32], F32, tag="Lt32")
                      for j in range(4):
                          nc.vector.transpose(Lt32[32 * j:32 * (j + 1), 0:32],
                                              psh[96:128, 32 * j:32 * (j + 1)])
                      nc.vector.tensor_add(Lsb[:], Lt32[:, 16:24], brB[:])
                      nc.vector.tensor_add(Lsb[:, 0:6], Lsb[:, 0:6],
                                           Lt32[:, 24:30])
                  else:
                      nc.vector.tensor_add(Lsb[:], psh[:, 112:120], brB[:])
                      nc.vector.tensor_add(Lsb[:, 0:6], Lsb[:, 0:6],
                                           psh[:, 120:126])
                  if XL8 and not XL0:
                      # xl router correction: fp8 product carries 2^15 * 2^11
                      nc.vector.scalar_tensor_tensor(
                          Lsb[:, 0:6], psh[:, 128:134], 2.0 ** -26,
                          Lsb[:, 0:6], op0=mybir.AluOpType.mult,
                          op1=mybir.AluOpType.add)
                  M8 = gate.tile([P, 8], F32, tag="M8")
                  nc.vector.max(out=M8[:], in_=Lsb[:])
                  nm1 = gate.tile([P, 1], F32, tag="nm1")
                  nc.vector.tensor_scalar_mul(nm1[:], M8[:, 0:1], -1.0)
                  es = gate.tile([P, 8], F32, tag="es")
                  ssum = gate.tile([P, 1], F32, tag="ssum")
                  nc.scalar.activation(es[:], Lsb[:], mybir.ActivationFunctionType.Exp,
                                       bias=nm1[:], accum_out=ssum[:])
                  rcp = gate.tile([P, 1], F32, tag="rcp")
                  nc.vector.reciprocal(rcp[:], ssum[:])
                  msk = gate.tile([P, 8], F32, tag="msk")
                  nc.vector.tensor_scalar(msk[:], Lsb[:], M8[:, 1:2], scalar2=None,
                                          op0=mybir.AluOpType.is_ge)

                  if TR1:
                      # stage 3': gate weights -> fp16, transpose on the DVE,
                      # expand to per-rank-row scales with a K=8 matmul, and
                      # scale H^T in place of the old PE transpose
                      wgt32 = gate.tile([P, 32], F16, tag="wgt32")
                      nc.vector.scalar_tensor_tensor(wgt32[:, 0:8], es[:],
                                                     rcp[:], msk[:],
                                                     op0=mybir.AluOpType.mult,
                                                     op1=mybir.AluOpType.mult)
                      # col 8 -> wgtT row 8 = 1.0 (scales the shared rows)
                      nc.vector.memset(wgt32[:, 8:9], 1.0)
                      wgtT = gate.tile([32, P], F16, tag="wgtT")
                      for j in range(4):
                          nc.vector.transpose(wgtT[0:32, 32 * j:32 * (j + 1)],
                                              wgt32[32 * j:32 * (j + 1), 0:32])
                      psS = psT.tile([112, P], F32)
                      nc.tensor.matmul(psS[:], EXPT[:], wgtT[0:9, :],
                                       start=True, stop=True)
                      Ssb = gate.tile([96, P], F16, tag="Ssb")
                      nc.vector.tensor_copy(Ssb[:], psS[0:96, :])
                      HsT = gate.tile([128, P], F16, tag="HsT")
                      nc.vector.tensor_mul(HsT[0:96, :], psh[0:96, 0:128],
                                           Ssb[:])
                      # rows 96:128: shared pass through (MASK), row 112
                      # becomes the constant 1.0 bias row (ONESROW)
                      nc.vector.tensor_mul(HsT[96:128, :],
                                           psh[96:128, 0:128], MASK[96:128, :])
                      nc.vector.tensor_add(HsT[96:128, :], HsT[96:128, :],
                                           ONESROW[96:128, :])
                  else:
                      wgt = gate.tile([P, 8], F32, tag="wgt")
                      nc.vector.scalar_tensor_tensor(wgt[:], es[:], rcp[:],
                                                     msk[:],
                                                     op0=mybir.AluOpType.mult,
                                                     op1=mybir.AluOpType.mult)

                      # stage 3: scale rank vectors by gate weights
                      sfull = gate.tile([P, 96], F32, tag="sfull")
                      for e in range(E):
                          nc.vector.tensor_copy(sfull[:, 16 * e:16 * (e + 1)],
                                                wgt[:, e:e + 1].to_broadcast([P, 16]))
                      Hs16 = gate.tile([P, 113], F16, tag="Hs16")
                      nc.vector.tensor_mul(Hs16[:, 0:96], psh[:, 0:96], sfull[:])
                      nc.vector.tensor_copy(Hs16[:, 96:112], psh[:, 96:112])
                      nc.vector.memset(Hs16[:, 112:113], 1.0)

                      # stage 4: transpose scaled rank vectors -> [113, 128] fp16
                      pst = psT.tile([113, P], F16)
                      nc.tensor.transpose(pst[:], Hs16[:], ident[:])
                      HsT = gate.tile([113, P], F16, tag="HsT")
                      nc.vector.tensor_copy(HsT[:], pst[:])

                  # stage 5b: adapter close + PSUM->SBUF copies + y DMA.
                  # With HSPL the close phase also carries h=1's base
                  # matmuls, so it must be emitted inline; otherwise defer
                  # it past the next tile's stage 1.
                  if HSPL:
                      s5_close(HsT)
                  else:
                      pending_close[0] = (lambda c=s5_close, h=HsT: c(h))
              if pending_close[0] is not None:
                  pending_close[0]()   # last tile's close, inside the rep body
                  pending_close[0] = None
    nc.compile()
    return nc


_cache: dict[int, bacc.Bacc] = {}


def _get_nc(T: int) -> bacc.Bacc:
    if T not in _cache:
        _cache[T] = build_kernel(T)
    return _cache[T]


def _pack_xT(xs: np.ndarray) -> np.ndarray:
    """[T, D] -> [T//P, P, D] with packed[t, p, kt*P + tok] = xs[t*P+tok, kt*P+p]."""
    TT = xs.shape[0] // P
    v = xs.reshape(TT, P, KT, P).transpose(0, 3, 2, 1)
    return np.ascontiguousarray(v).reshape(TT, P, D)


def _pack_x8(xs32: np.ndarray) -> np.ndarray:
    """fp32 [T, D] -> e4m3 [T//P, P, NK8//2, 2, P] DoubleRow stationary.

    Plain layout: x8[t, p, kp, half, tok]
                    = e4m3(x[128t + tok, D16 + 256 kp + 128 half + p])
    SwInterleave: the two k-half planes are interleaved per column with
    columns reversed: stored[t, p, kp, 2j + half] = plain[..., half, 127-j]"""
    import ml_dtypes
    TT = xs32.shape[0] // P
    v = xs32[:, D16:].reshape(TT, P, NK8 // 2, 2, P)
    v = v.transpose(0, 4, 2, 3, 1)          # t, p, kp, half, tok
    if SWI:
        v = v[..., ::-1]                    # reverse tok
        v = v.transpose(0, 1, 2, 4, 3)      # t, p, kp, tok_r, half
        v = v.reshape(TT, P, NK8 // 2, 2, P)
    return np.ascontiguousarray(v).astype(ml_dtypes.float8_e4m3)


def kernel(**inputs: np.ndarray) -> np.ndarray:
    import ml_dtypes
    x = np.ascontiguousarray(np.asarray(inputs["x"], dtype=np.float32).reshape(NTOK, D))
    T = NTOK // NCORES
    xh = x.astype(np.float16)
    if XL0:
        xl = None
    else:
        xl32 = x - xh.astype(np.float32)
        if XL8:
            xl = np.clip(xl32 * 2.0 ** 15, -240.0, 240.0).astype(
                ml_dtypes.float8_e4m3)
        else:
            xl = xl32.astype(np.float16)
    shards = [(xh[i * T:(i + 1) * T],
               None if XL0 else xl[i * T:(i + 1) * T],
               x[i * T:(i + 1) * T]) for i in range(NCORES)]
    W = np.asarray(inputs["W_base"], dtype=np.float32)
    Wt = (64.0 * W[D16:]).reshape(NK8 // 2, 2, P, D)
    common = {
        "W16": np.ascontiguousarray((64.0 * W[:D16]).astype(np.float16)),
        "W8": np.ascontiguousarray(Wt.transpose(0, 2, 1, 3)).astype(
            ml_dtypes.float8_e4m3),
        "b_base": 64.0 * np.ascontiguousarray(
            inputs["b_base"], dtype=np.float32).reshape(1, D),
        "A_s": np.ascontiguousarray(inputs["A_s"], dtype=np.float32),
        "B_s": 64.0 * np.ascontiguousarray(inputs["B_s"], dtype=np.float32),
        "A_r": np.ascontiguousarray(inputs["A_r"], dtype=np.float32),
        "B_r": 64.0 * np.ascontiguousarray(inputs["B_r"], dtype=np.float32),
        "W_router": np.ascontiguousarray(inputs["W_router"], dtype=np.float32),
        "b_router": np.ascontiguousarray(inputs["b_router"], dtype=np.float32).reshape(1, E),
    }
    if TR1:
        expt = np.zeros((9, 112), dtype=np.float16)
        for e in range(E):
            expt[e, 16 * e:16 * (e + 1)] = 1.0
        expt[8, 96:112] = 1.0
        common["EXPT"] = expt
        onesrow = np.zeros((32, P), dtype=np.float16)
        onesrow[16, :] = 1.0          # partition 96+16 = 112: the bias row
        common["onesrow"] = onesrow
    xlkey = "xl8T" if XL8 else "xlT"
    in_maps = [dict(common,
                    **{"xhT": _pack_xT(sh), "x8T": _pack_x8(sf)},
                    **({} if XL0 else {xlkey: _pack_xT(sl)}))
               for sh, sl, sf in shards]
    nc = _get_nc(T)
    res = bass_utils.run_bass_kernel_spmd(nc, in_maps, core_ids=list(range(NCORES)))
    out = np.concatenate([res.results[i]["y"].astype(np.float32)
                          for i in range(NCORES)], axis=0)
    return out.reshape(B, S, D)


if __name__ == "__main__":
    rng = np.random.default_rng(0)
    demo = {
        "x": rng.standard_normal((B, S, D), dtype=np.float32),
        "W_base": 0.02 * rng.standard_normal((D, D), dtype=np.float32),
        "b_base": 0.02 * rng.standard_normal((D,), dtype=np.float32),
        "A_s": 0.02 * rng.standard_normal((D, R), dtype=np.float32),
        "B_s": 0.02 * rng.standard_normal((R, D), dtype=np.float32),
        "A_r": 0.02 * rng.standard_normal((E, D, R), dtype=np.float32),
        "B_r": 0.02 * rng.standard_normal((E, R, D), dtype=np.float32),
        "W_router": 0.02 * rng.standard_normal((D, E), dtype=np.float32),
        "b_router": 0.02 * rng.standard_normal((E,), dtype=np.float32),
    }
    y = kernel(**demo)
    print("kernel ran, output", y.shape, y.dtype)



# revision 79
# speedup vs baseline: 1.0362x; 1.0362x over previous
"""Trainium2 Bass kernel for the MoE-Adapter module.

Math (per token):
  y = x @ W_base + b_base                       (dense base, stop-grad)
    + (x @ A_s) @ B_s                           (shared rank-16 LoRA)
    + sum_e w_e * (x @ A_r[e]) @ B_r[e]         (6 routed rank-16 LoRA experts)
  w = softmax(x @ W_router + b_router) masked to its top-2 entries

Strategy: data-parallel over the 16384 tokens across 8 NeuronCores (2048
tokens/core); all weights replicated.  Compute in fp16 (full PE rate,
fp32 PSUM accumulation).  The router is computed in fp16-pair precision
(x = xh + xl, W_router = Wrh + Wrl, logits = xh@Wrh + xh@Wrl + xl@Wrh)
so top-2 selection matches the fp32 reference.

Per-core layout: the host stages x as an fp16 pair in transposed,
token-tile-major layout (xhT/xlT: [tile, din%128, ktile*128+tok]) and
W_base as fp16, so every device-side load is one contiguous DMA (the
HWDGE direct2d descriptor only has room for 2 sync-wait commands, which
rules out multi-dependency staging chains on-chip).  The rank-space
projections of all 7 experts plus both router column groups are packed
into one 128-wide rhs so the whole adapter+router down-projection is a
single accumulation chain.  Gate weights are applied in rank space, the
scaled rank vectors are transposed on the PE and folded into the base
matmul's PSUM accumulation together with the bias (K=1 ones matmul).

The last NK8=4 k-tiles of the base matmul run in fp8 e4m3 with
perf_mode=DoubleRow (2 fp8 MACs/PE-cell/cycle): x is quantized host-side
straight from fp32, W's tail is quantized at 64x so its values sit in
e4m3's normal range, and the entire Y accumulation runs at 64x (W16, B_r,
B_s, b_base are host-scaled) so fp16 and fp8 partial products share one
PSUM group; the PSUM->SBUF copy applies the 1/64.  Four k-tiles is the
most the 2e-2 abs-max error budget allows (measured rel err 0.0178, vs
0.0003 all-fp16; the error is ~100% fp8 quantization, split evenly
between the x and W sides, so NK8=6 would land at 0.0218).

Measured refinements over the original baseline (297.7us -> ~285us on
the same measurement setup):
 - fp8 DoubleRow matmuls stream the full 512-col psum bank (FP8W): the
   512-cycle stream covers the next matmul's ~213ns no-FWL LDWEIGHTS via
   the background-weight-buffer ping-pong.  Ablation: the fp8 tail now
   costs ~135ns/MM vs ~216ns for an equivalent fp16 MM (fp8 saves 37us
   vs an all-fp16 tail).
 - y leaves the device as fp16 (Y16) and xl as e4m3 at 2^15 scale (XL8,
   own 2^26-scaled psum columns; start=False always since a start=True
   matmul clears the WHOLE psum bank's has_written bits - measured).
 - the gating-independent base matmuls are emitted before the gating
   chain so the serial DVE/ACT chain overlaps them (s5_open/s5_close).
 - DMA is fully hidden (noxdma+noydma ablation: no change); stage-1
   costs its exact PE streaming time (20us of 290); the kernel is
   PE-streaming-bound, so fp16 1-col/cycle sets the floor.
Dead ends (measured): dropping the xl router correction flips top-2 for
1/16384 tokens and lands rel err at 0.0231 (over budget); computing
stage 1 transposed (TR1) to kill the PE transpose measured +1.4us.
"""

import os
import sys

import numpy as np

for _p in ("/opt/trn_rl_repo",):
    if os.path.isdir(_p) and _p not in sys.path:
        sys.path.insert(0, _p)

import concourse.bass as bass
import concourse.mybir as mybir
import concourse.tile as tile
from concourse import bacc
from concourse import bass_utils
from concourse.masks import make_identity

B, S, D, E, R = 4, 4096, 2048, 6, 16
NCORES = 8
NTOK = B * S               # 16384 tokens total
P = 128
KT = D // P                # 16 k-tiles over the contraction dim
NK8 = 4                    # trailing k-tiles done in fp8 e4m3 DoubleRow
KT16 = KT - NK8            # leading k-tiles done in fp16
D16 = KT16 * P             # contraction split point
NCHUNK = 512               # PSUM bank width (fp32)
NCH = D // NCHUNK          # 4 output column chunks
NEG = -60000.0             # exp() flushes this to 0; fits in fp16

F32 = mybir.dt.float32
F16 = mybir.dt.float16
F8 = mybir.dt.float8e4
SWI = False                # DoubleRowSwInterleave: measured equal to DoubleRow
ILV = False                # interleaving fp8 between fp16 k-tiles measured
                           # +11us/rep WORSE than the block order (breaks the
                           # fp16 FWL prefetch chain); keep the fp8 block
CONS = True                # one fp8 block per tile (fewer FWL<->DoubleRow
                           # mode switches); measured ~-6us/rep vs split
XLI = True                 # interleave xl router matmuls between xh k-tiles:
                           # hides their LDWEIGHTS, measured -7us/rep
ABLATE = frozenset()       # timing-probe ablations (break correctness):
                           # "noxdma" (x DMAs only for t==0), "noydma",
                           # "nostage1" (skip adapter/router/gating),
                           # "nofp8" (tail k-tiles as fp16 on reused planes)
HSPL = True                # pipeline the two 1024-col halves: close+copy of
                           # h=0 overlaps h=1's base matmuls, freeing PSUM
                           # banks mid-tile (kills the per-tile bank stall)
XL8 = True                 # xl residual travels as e4m3 (halves xl DMA)
XL0 = False                # drop the xl router correction: MEASURED OVER
                           # BUDGET (rel 0.0231) -- the 1/16384 top-2 flip
                           # lands where the fp8 error is already large
FP8W = True                # fp8 base matmuls stream 512 cols (hides LDW)
Y16 = True                 # y leaves the device as fp16 (halves out DMA)
TR1 = False                # stage 1 computes H^T directly (stationary=AR,
                           # moving=xhT), killing the PE transpose; gating
                           # runs off small DVE 32x32 block transposes and
                           # the gate scales fold in via a K=8 expand matmul


def _stage5_parts(nc, tc, psY, yout, Wk, W8sb, x8sb, Bc, y_d, t, xhT,
                  nchunk, nch):
    """Base matmul + adapter up-projection (incl. bias row), fused in PSUM.

    Returns (emit_open, emit_close_all): open emits the gating-INDEPENDENT
    fp16+fp8 base matmuls; close_all(HsT) emits the adapter close matmuls,
    PSUM->SBUF copies and the y DMA.  Splitting them lets the ~1.5-2.5us
    serial DVE/ACT gating chain run under the ~11us of base matmuls instead
    of stalling the PE at the first gating-dependent instruction (the PE
    executes its queue in emission order).

    The whole Y accumulation runs at 64x natural scale (W16/W8/Bc/bias are
    host-scaled by 64) so the fp8 tail k-tiles land in e4m3's normal range;
    the PSUM->SBUF copy applies the 1/64."""
    ysb = yout.tile([P, D], F16 if Y16 else F32, tag="ysb", name=f"ysb_{t}")
    pm = (mybir.MatmulPerfMode.DoubleRowSwInterleave if SWI
          else mybir.MatmulPerfMode.DoubleRow)
    hs = list(range(nch // 2))
    psys = {h: [psY.tile([P, nchunk], F32, tag=f"psy{n}",
                         name=f"psy{h}_{n}_{t}") for n in range(2)]
            for h in hs}
    # The NK8 trailing k-tiles run in fp8 e4m3 DoubleRow: each instr loads
    # two full 128x128 stationary planes (one per k-tile of the pair) and
    # streams the full 512-col psum bank of the paired W8 rows.  512-wide
    # streams (241ns) cover the 213ns no-FWL LDWEIGHTS of the NEXT matmul
    # (background weight buffer ping-pong), so consecutive jobs alternate
    # the k-pair.  The fp8 matmuls stay in back-to-back blocks: interleaving
    # them with fp16 matmuls measured +11us/rep worse (breaks the fp16 FWL
    # prefetch chain).
    if FP8W:
        fp8_jobs = [(n, kp, 0) for n in range(2) for kp in range(NK8 // 2)]
    else:
        fp8_jobs = [(j % 2, (j // 2) % 2, j // 4) for j in range(2 * NK8)]
    fp8_w = nchunk if FP8W else 256

    def emit_fp16(h):
        for kt in range(KT16):
            for n in range(2):
                lo = (2 * h + n) * nchunk
                nc.tensor.matmul(psys[h][n][:], xhT[:, kt * P:(kt + 1) * P],
                                 Wk[kt][:, lo:lo + nchunk],
                                 start=(kt == 0), stop=False)

    def emit_fp8(h, job):
        n, kp, ch = job
        lo = (2 * h + n) * nchunk
        if "nofp8" in ABLATE:
            for kk in range(2):
                kt = 2 * kp + kk
                nc.tensor.matmul(psys[h][n][:],
                                 xhT[:, (KT16 + kt) * P:(KT16 + kt + 1) * P],
                                 Wk[kt][:, lo:lo + nchunk],
                                 start=False, stop=False,
                                 skip_group_check=True)
            return
        c0 = fp8_w * ch
        nc.tensor.matmul(
            psys[h][n][:, c0:c0 + fp8_w],
            x8sb[:, kp, :, :],
            W8sb[kp][:, :, lo + c0:lo + c0 + fp8_w],
            start=False, stop=False,
            perf_mode=pm,
            skip_group_check=True)

    def emit_close(h, HsT):
        for n in range(2):
            lo = (2 * h + n) * nchunk
            nc.tensor.matmul(psys[h][n][:], HsT[0:113],
                             Bc[0:113, lo:lo + nchunk],
                             start=False, stop=True)
            # PSUM->SBUF drain on the DVE, NOT the ACT: the ACT FIFO runs
            # the gating exp, and 4 queued copies ahead of it delayed the
            # whole gating->HsT->close chain by ~3us/tile (sim)
            nc.vector.tensor_scalar_mul(ysb[:, lo:lo + nchunk],
                                        psys[h][n][:], 1.0 / 64.0)

    def _ydma():
        if "noydma" in ABLATE:
            nc.sync.dma_start(y_d[t * P:t * P + 1, :], ysb[0:1, :])
        else:
            nc.sync.dma_start(y_d[t * P:(t + 1) * P, :], ysb[:])

    if HSPL:
        # column-half pipelining: h=0's base+fp8 in the open phase; its
        # close + copies drain that half's PSUM banks while h=1's base
        # matmuls run, so the NEXT tile's h=0 matmuls never wait on a bank
        # (the bank-recycle stall also re-throttled the PE p-state)
        def emit_open():
            emit_fp16(hs[0])
            for job in fp8_jobs:
                emit_fp8(hs[0], job)

        def emit_close_all(HsT):
            emit_close(hs[0], HsT)
            for h in hs[1:]:
                emit_fp16(h)
                for job in fp8_jobs:
                    emit_fp8(h, job)

            # close_h1 + its DVE copies + the y DMA are DEFERRED to just
            # after the next tile's stage 1, so the bank-release ticks the
            # next tile's base_h1 waits on resolve before that tile's
            # gating chain (avoids a coarsened-wait false dependency)
            def tail():
                for h in hs[1:]:
                    emit_close(h, HsT)
                _ydma()
            return tail
    else:
        def emit_open():
            # one fp8 block per tile: fewer FWL<->DoubleRow mode switches
            for h in hs:
                emit_fp16(h)
            for h in hs:
                for job in fp8_jobs:
                    emit_fp8(h, job)

        def emit_close_all(HsT):
            for h in hs:
                emit_close(h, HsT)
            _ydma()

    return emit_open, emit_close_all


def build_kernel(T: int, repeat: int = 1, nchunk: int = NCHUNK,
                 depths: tuple = (6, 4, 4)) -> bacc.Bacc:
    """Build the per-core kernel for T tokens (T % 128 == 0).

    repeat > 1 wraps the main loop in a device-side For_i that redoes the
    whole computation; used only for wall-clock timing (amplifies kernel
    time far above the dispatch noise)."""
    assert XL8 or not TR1, "TR1 needs the XL8 xl layout (separate psum cols)"
    TT = T // P
    nch = D // nchunk
    xt_b, gate_b, yout_b = depths
    nc = bacc.Bacc("TRN2", target_bir_lowering=False, debug=False)

    xhT_d = nc.dram_tensor("xhT", [T // P, P, D], F16, kind="ExternalInput").ap()
    # xl (the fp16 residual of x, used only for the router's low-order
    # logit correction) travels as e4m3 at 2^15 scale: |xl| <= 2.5e-3 so
    # 2^15*xl <= ~83 sits in e4m3's normal range; its 2^-4 relative quant
    # error perturbs logits by ~5e-6, far below the ~1e-4 logit gaps that
    # matter for top-2 selection.  Halves the xl DMA traffic vs fp16.
    xlT_d = (None if XL0 else
             nc.dram_tensor("xl8T" if XL8 else "xlT", [T // P, P, D],
                            F8 if XL8 else F16, kind="ExternalInput").ap())
    x8_d = (nc.dram_tensor("x8T", [T // P, P, NK8 // 2, 2, P], F8,
                           kind="ExternalInput").ap() if NK8 else None)
    Wb_d = nc.dram_tensor("W16", [D16, D], F16, kind="ExternalInput").ap()
    W8_d = (nc.dram_tensor("W8", [NK8 // 2, P, 2, D], F8,
                           kind="ExternalInput").ap() if NK8 else None)
    bb_d = nc.dram_tensor("b_base", [1, D], F32, kind="ExternalInput").ap()
    As_d = nc.dram_tensor("A_s", [D, R], F32, kind="ExternalInput").ap()
    Bs_d = nc.dram_tensor("B_s", [R, D], F32, kind="ExternalInput").ap()
    Ar_d = nc.dram_tensor("A_r", [E, D, R], F32, kind="ExternalInput").ap()
    Br_d = nc.dram_tensor("B_r", [E, R, D], F32, kind="ExternalInput").ap()
    Wr_d = nc.dram_tensor("W_router", [D, E], F32, kind="ExternalInput").ap()
    br_d = nc.dram_tensor("b_router", [1, E], F32, kind="ExternalInput").ap()
    expt_d = ones_d = None
    if TR1:
        expt_d = nc.dram_tensor("EXPT", [9, 112], F16, kind="ExternalInput").ap()
        ones_d = nc.dram_tensor("onesrow", [32, P], F16, kind="ExternalInput").ap()
    # y leaves the device as fp16 (values <= ~6, fp16 rounding adds <= 4e-4
    # relative error); the host upcasts.  Halves the output DMA traffic.
    y_d = nc.dram_tensor("y", [T, D], F16, kind="ExternalOutput").ap()

    with tile.TileContext(nc) as tc:
        with (
            tc.tile_pool(name="const", bufs=1) as const,
            tc.tile_pool(name="wpool", bufs=1) as wpool,
            tc.tile_pool(name="small", bufs=1) as small,
            tc.tile_pool(name="xT", bufs=xt_b) as xTp,
            tc.tile_pool(name="gate", bufs=gate_b) as gate,
            tc.tile_pool(name="yout", bufs=yout_b) as yout,
            tc.tile_pool(name="psY", bufs=(2 if nchunk <= 512 else 1),
                         space="PSUM") as psY,
            tc.tile_pool(name="psH", bufs=2, space="PSUM") as psH,
            tc.tile_pool(name="psT", bufs=2, space="PSUM") as psT,
        ):
            # ---- constants ----
            ones = const.tile([1, P], F16)
            nc.vector.memset(ones[:], 1.0)
            ident = const.tile([P, P], F16)
            make_identity(nc, ident[:])

            # brow: K=1 bias row for the adapter/router chain
            # [0]*112 | b_router (6) | NEG pads (2) | 0 (8)
            brow = const.tile([1, P], F16)
            nc.vector.memset(brow[:], 0.0)
            brs = small.tile([1, E], F32, tag="brs")
            nc.sync.dma_start(brs[:], br_d[:])
            nc.vector.tensor_copy(brow[:, 112:118], brs[:])
            nc.vector.memset(brow[:, 118:120], NEG)

            # broadcast the router-bias row to all 128 partitions once (K=1
            # ones matmul); the per-tile Lsb extraction adds it on the DVE,
            # replacing a per-tile 128-col PE matmul
            psh = psH.tile([P, P], F32)
            nc.tensor.matmul(psh[:, 0:8], ones[:], brow[:, 112:120],
                             start=True, stop=True)
            brB = const.tile([P, 8], F32)
            nc.vector.tensor_copy(brB[:], psh[:, 0:8])

            # b_base is folded into the adapter up-projection as rank row
            # 112 (its HsT row is a constant 1.0 column in rank space)

            # EXPT (DMA'd host const — compute engines can't write at
            # unaligned partition bases): EXPT[e, 16e:16e+16] = 1 for the 6
            # experts, EXPT[8, 96:112] = 1 so the shared rows scale by the
            # constant-1 row of wgtT.  One K=9 matmul expands wgtT [9, tok]
            # to per-rank-row scales S^T [112, tok].
            EXPT = None
            if TR1:
                EXPT = const.tile([9, 112], F16)
                nc.sync.dma_start(EXPT[:], expt_d[:])
                # MASK/ONESROW for HsT rows 96:128: shared rows pass through,
                # row 112 becomes the constant bias row, 113:128 zero.  Both
                # live at partitions 96:128 of full-height tiles (engine ops
                # need equal base partitions across SBUF operands; ONESROW
                # is DMA'd since compute can't write at partition 112).
                MASK = const.tile([128, P], F16)
                nc.vector.memset(MASK[96:128, :], 0.0)
                nc.vector.memset(MASK[96:112, :], 1.0)
                ONESROW = const.tile([128, P], F16)
                nc.sync.dma_start(ONESROW[96:128, :], ones_d[:])

            # ---- adapter down-proj + router rhs: [P, KT, 128] fp16 ----
            # cols: A_r[e]*6 (96) | A_s(16) | Wrh(6) | 0(2) | Wrl(6) | 0(2)
            AR = const.tile([P, KT, P], F16)
            nc.vector.memset(AR[:, :, 118:120], 0.0)
            nc.vector.memset(AR[:, :, 126:128], 0.0)
            for e in range(E):
                art = small.tile([P, KT, R], F32, tag=f"art{e}")
                nc.sync.dma_start(art[:], Ar_d[e].rearrange("(k p) r -> p k r", p=P))
                nc.vector.tensor_copy(AR[:, :, 16 * e:16 * (e + 1)], art[:])
            ast = small.tile([P, KT, R], F32, tag="ast")
            nc.sync.dma_start(ast[:], As_d.rearrange("(k p) r -> p k r", p=P))
            nc.vector.tensor_copy(AR[:, :, 96:112], ast[:])
            wrt = small.tile([P, KT, E], F32, tag="wrt")
            nc.sync.dma_start(wrt[:], Wr_d.rearrange("(k p) e -> p k e", p=P))
            nc.vector.tensor_copy(AR[:, :, 112:118], wrt[:])
            # Wrl = fp32(Wr) - fp16(Wr), rounded to fp16
            nc.vector.tensor_sub(AR[:, :, 120:126], wrt[:], AR[:, :, 112:118])
            # e4m3 copy of Wrh at 2^11 scale (max |Wrh| ~ 0.09 -> ~190, in
            # range) for the fp8 xl router matmuls; the 1.8% quant error
            # only multiplies the tiny xl residual, so it is negligible.
            ARl8 = None
            if XL8:
                ARl8 = const.tile([P, KT, 8], F8)
                nc.vector.memset(ARl8[:, :, 6:8], 0.0)
                nc.scalar.activation(ARl8[:, :, 0:6], AR[:, :, 112:118],
                                     mybir.ActivationFunctionType.Copy,
                                     scale=2048.0)

            # ---- up-proj weights + bias row: rows 0:96 = B_r, 96:112 =
            # B_s, 112 = b_base (the matching HsT row is constant 1.0).
            # Rows 113:128 are zero padding, never read (K=113); compute-
            # engine partition bases must be 32-aligned, hence the [96:128)
            # staging block.
            Bc = const.tile([128, D], F16)
            bst = small.tile([96, D], F32, tag="bst")
            nc.sync.dma_start(bst[:], Br_d.rearrange("e r d -> (e r) d"))
            nc.scalar.activation(Bc[0:96, :], bst[:],
                                 mybir.ActivationFunctionType.Copy)
            bst2 = small.tile([32, D], F32, tag="bst2")
            nc.vector.memset(bst2[:], 0.0)
            nc.sync.dma_start(bst2[0:16, :], Bs_d[:])
            nc.sync.dma_start(bst2[16:17, :], bb_d[:])
            nc.scalar.activation(Bc[96:128, :], bst2[:],
                                 mybir.ActivationFunctionType.Copy)

            # ---- base weight fp16 k-tiles (host pre-cast to fp16, x64) ----
            Wk = []
            for kt in range(KT16):
                wk = wpool.tile([P, D], F16, tag=f"w{kt}")
                nc.sync.dma_start(wk[:], Wb_d[kt * P:(kt + 1) * P, :])
                Wk.append(wk)
            # trailing k-tiles as fp8 e4m3 (x64), packed [p, half, col],
            # one tile per k-pair (keeps slots the same 4KB as wk tiles)
            W8sb = []
            for kp in range(NK8 // 2):
                w8 = wpool.tile([P, 2, D], F8, tag=f"w8_{kp}")
                nc.sync.dma_start(w8[:], W8_d[kp])
                W8sb.append(w8)

            hst_const = None
            if "nostage1" in ABLATE:
                hst_const = const.tile([113, P], F16)
                nc.vector.memset(hst_const[:], 0.5)

            # ---- main loop over 128-token tiles ----
            import contextlib
            rep_ctx = (tc.For_i(0, repeat, 1) if repeat > 1
                       else contextlib.nullcontext())
            with rep_ctx:
              # tile t's close matmuls are deferred until after tile t+1's
              # stage-1 matmuls so the close's wait on the HsT DVE copy is
              # covered by extra PE work (sim: removes ~0.7us/tile PE gap)
              pending_close = [None]
              for t in range(TT):
                  thin = "noxdma" in ABLATE and t > 0
                  xhT = xTp.tile([P, D], F16, tag="xhT")
                  nc.sync.dma_start(xhT[0:1, :] if thin else xhT[:],
                                    xhT_d[t, 0:1] if thin else xhT_d[t])
                  xlT = None
                  if not XL0:
                      xlT = xTp.tile([P, D], F8 if XL8 else F16, tag="xlT")
                      nc.sync.dma_start(xlT[0:1, :] if thin else xlT[:],
                                        xlT_d[t, 0:1] if thin else xlT_d[t])
                  x8sb = None
                  if NK8:
                      x8sb = xTp.tile([P, NK8 // 2, 2, P], F8, tag="x8T")
                      nc.sync.dma_start(x8sb[0:1] if thin else x8sb[:],
                                        x8_d[t, 0:1] if thin else x8_d[t])

                  # stage 1: rank-space projections + router logits.  The
                  # 6-col xl matmuls (fp8, own 2^26-scaled accumulation in
                  # cols 128:134 of the same psum bank) are interleaved
                  # between the 128-col xh matmuls (XLI) so each xl
                  # LDWEIGHTS hides under an xh stream; back-to-back 6-col
                  # matmuls are load-serialized.
                  if "nostage1" in ABLATE:
                      op, cl = _stage5_parts(nc, tc, psY, yout, Wk, W8sb,
                                             x8sb, Bc, y_d, t, xhT,
                                             nchunk, nch)
                      op()
                      cl(hst_const)
                      continue
                  psh = psH.tile([P, 136], F32)

                  def emit_xl(kt):
                      if XL0:
                          return
                      if XL8:
                          # start=False always: the xh kt0 matmul's
                          # start=True clears the WHOLE bank (measured), so
                          # cols 128:134 begin cleared and the first write
                          # lands fresh; start=True here would wipe the xh
                          # kt0 partial.
                          nc.tensor.matmul(psh[:, 128:134],
                                           xlT[:, kt * P:(kt + 1) * P],
                                           ARl8[:, kt, 0:6], start=False,
                                           stop=(kt == KT - 1),
                                           skip_group_check=True)
                      else:
                          nc.tensor.matmul(psh[:, 112:118],
                                           xlT[:, kt * P:(kt + 1) * P],
                                           AR[:, kt, 112:118], start=False,
                                           stop=(kt == KT - 1),
                                           skip_group_check=True)

                  def emit_xh(kt):
                      if TR1:
                          # H^T [rank, tok]: stationary=AR tile, moving=xhT
                          nc.tensor.matmul(psh[:, 0:128], AR[:, kt, :],
                                           xhT[:, kt * P:(kt + 1) * P],
                                           start=(kt == 0), stop=False)
                      else:
                          nc.tensor.matmul(psh[:, 0:128],
                                           xhT[:, kt * P:(kt + 1) * P],
                                           AR[:, kt, :], start=(kt == 0),
                                           stop=False)

                  if XLI:
                      for kt in range(KT):
                          emit_xh(kt)
                          emit_xl(kt)
                  else:
                      for kt in range(KT):
                          emit_xh(kt)
                      for kt in range(KT):
                          emit_xl(kt)

                  # deferred close of the PREVIOUS tile: its gating/HsT chain
                  # is long done (covered by the previous tile's base matmuls
                  # plus this tile's stage 1)
                  if pending_close[0] is not None:
                      pending_close[0]()
                      pending_close[0] = None

                  # stage 5a: the gating-independent base fp16+fp8 matmuls go
                  # into the PE queue NOW, so the serial gating chain below
                  # (DVE/ACT, ~2us) overlaps them instead of stalling the PE
                  s5_open, s5_close = _stage5_parts(nc, tc, psY, yout, Wk,
                                                    W8sb, x8sb, Bc, y_d, t,
                                                    xhT, nchunk, nch)
                  s5_open()

                  # stage 2: top-2 gating  w = softmax(L) * (L >= secondmax(L))
                  Lsb = gate.tile([P, 8], F32, tag="Lsb")
                  if TR1:
                      # H^T rows 96:128 hold [shared(16) | Wrh logits(6) |
                      # 0(2) | Wrl logits(6) | 0(2)]^T; DVE 32x32 block
                      # transposes bring the router rows into token-major
                      Lt32 = gate.tile([P, 32], F32, tag="Lt32")
                      for j in range(4):
                          nc.vector.transpose(Lt32[32 * j:32 * (j + 1), 0:32],
                                              psh[96:128, 32 * j:32 * (j + 1)])
                      nc.vector.tensor_add(Lsb[:], Lt32[:, 16:24], brB[:])
                      nc.vector.tensor_add(Lsb[:, 0:6], Lsb[:, 0:6],
                                           Lt32[:, 24:30])
                  else:
                      nc.vector.tensor_add(Lsb[:], psh[:, 112:120], brB[:])
                      nc.vector.tensor_add(Lsb[:, 0:6], Lsb[:, 0:6],
                                           psh[:, 120:126])
                  if XL8 and not XL0:
                      # xl router correction: fp8 product carries 2^15 * 2^11
                      nc.vector.scalar_tensor_tensor(
                          Lsb[:, 0:6], psh[:, 128:134], 2.0 ** -26,
                          Lsb[:, 0:6], op0=mybir.AluOpType.mult,
                          op1=mybir.AluOpType.add)
                  M8 = gate.tile([P, 8], F32, tag="M8")
                  nc.vector.max(out=M8[:], in_=Lsb[:])
                  nm1 = gate.tile([P, 1], F32, tag="nm1")
                  nc.vector.tensor_scalar_mul(nm1[:], M8[:, 0:1], -1.0)
                  es = gate.tile([P, 8], F32, tag="es")
                  ssum = gate.tile([P, 1], F32, tag="ssum")
                  nc.scalar.activation(es[:], Lsb[:], mybir.ActivationFunctionType.Exp,
                                       bias=nm1[:], accum_out=ssum[:])
                  rcp = gate.tile([P, 1], F32, tag="rcp")
                  nc.vector.reciprocal(rcp[:], ssum[:])
                  msk = gate.tile([P, 8], F32, tag="msk")
                  nc.vector.tensor_scalar(msk[:], Lsb[:], M8[:, 1:2], scalar2=None,
                                          op0=mybir.AluOpType.is_ge)

                  if TR1:
                      # stage 3': gate weights -> fp16, transpose on the DVE,
                      # expand to per-rank-row scales with a K=8 matmul, and
                      # scale H^T in place of the old PE transpose
                      wgt32 = gate.tile([P, 32], F16, tag="wgt32")
                      nc.vector.scalar_tensor_tensor(wgt32[:, 0:8], es[:],
                                                     rcp[:], msk[:],
                                                     op0=mybir.AluOpType.mult,
                                                     op1=mybir.AluOpType.mult)
                      # col 8 -> wgtT row 8 = 1.0 (scales the shared rows)
                      nc.vector.memset(wgt32[:, 8:9], 1.0)
                      wgtT = gate.tile([32, P], F16, tag="wgtT")
                      for j in range(4):
                          nc.vector.transpose(wgtT[0:32, 32 * j:32 * (j + 1)],
                                              wgt32[32 * j:32 * (j + 1), 0:32])
                      psS = psT.tile([112, P], F32)
                      nc.tensor.matmul(psS[:], EXPT[:], wgtT[0:9, :],
                                       start=True, stop=True)
                      Ssb = gate.tile([96, P], F16, tag="Ssb")
                      nc.vector.tensor_copy(Ssb[:], psS[0:96, :])
                      HsT = gate.tile([128, P], F16, tag="HsT")
                      nc.vector.tensor_mul(HsT[0:96, :], psh[0:96, 0:128],
                                           Ssb[:])
                      # rows 96:128: shared pass through (MASK), row 112
                      # becomes the constant 1.0 bias row (ONESROW)
                      nc.vector.tensor_mul(HsT[96:128, :],
                                           psh[96:128, 0:128], MASK[96:128, :])
                      nc.vector.tensor_add(HsT[96:128, :], HsT[96:128, :],
                                           ONESROW[96:128, :])
                  else:
                      wgt = gate.tile([P, 8], F32, tag="wgt")
                      nc.vector.scalar_tensor_tensor(wgt[:], es[:], rcp[:],
                                                     msk[:],
                                                     op0=mybir.AluOpType.mult,
                                                     op1=mybir.AluOpType.mult)

                      # stage 3: scale rank vectors by gate weights
                      sfull = gate.tile([P, 96], F32, tag="sfull")
                      for e in range(E):
                          nc.vector.tensor_copy(sfull[:, 16 * e:16 * (e + 1)],
                                                wgt[:, e:e + 1].to_broadcast([P, 16]))
                      Hs16 = gate.tile([P, 113], F16, tag="Hs16")
                      nc.vector.tensor_mul(Hs16[:, 0:96], psh[:, 0:96], sfull[:])
                      nc.vector.tensor_copy(Hs16[:, 96:112], psh[:, 96:112])
                      nc.vector.memset(Hs16[:, 112:113], 1.0)

                      # stage 4: transpose scaled rank vectors -> [113, 128] fp16
                      pst = psT.tile([113, P], F16)
                      nc.tensor.transpose(pst[:], Hs16[:], ident[:])
                      HsT = gate.tile([113, P], F16, tag="HsT")
                      nc.vector.tensor_copy(HsT[:], pst[:])

                  # stage 5b: adapter close + PSUM->SBUF copies + y DMA.
                  # With HSPL the close phase also carries h=1's base
                  # matmuls, so it must be emitted inline; otherwise defer
                  # it past the next tile's stage 1.
                  if HSPL:
                      pending_close[0] = s5_close(HsT)   # returns the tail
                  else:
                      pending_close[0] = (lambda c=s5_close, h=HsT: c(h))
              if pending_close[0] is not None:
                  pending_close[0]()   # last tile's close, inside the rep body
                  pending_close[0] = None
    nc.compile()
    return nc


_cache: dict[int, bacc.Bacc] = {}


def _get_nc(T: int) -> bacc.Bacc:
    if T not in _cache:
        _cache[T] = build_kernel(T)
    return _cache[T]


def _pack_xT(xs: np.ndarray) -> np.ndarray:
    """[T, D] -> [T//P, P, D] with packed[t, p, kt*P + tok] = xs[t*P+tok, kt*P+p]."""
    TT = xs.shape[0] // P
    v = xs.reshape(TT, P, KT, P).transpose(0, 3, 2, 1)
    return np.ascontiguousarray(v).reshape(TT, P, D)


def _pack_x8(xs32: np.ndarray) -> np.ndarray:
    """fp32 [T, D] -> e4m3 [T//P, P, NK8//2, 2, P] DoubleRow stationary.

    Plain layout: x8[t, p, kp, half, tok]
                    = e4m3(x[128t + tok, D16 + 256 kp + 128 half + p])
    SwInterleave: the two k-half planes are interleaved per column with
    columns reversed: stored[t, p, kp, 2j + half] = plain[..., half, 127-j]"""
    import ml_dtypes
    TT = xs32.shape[0] // P
    v = xs32[:, D16:].reshape(TT, P, NK8 // 2, 2, P)
    v = v.transpose(0, 4, 2, 3, 1)          # t, p, kp, half, tok
    if SWI:
        v = v[..., ::-1]                    # reverse tok
        v = v.transpose(0, 1, 2, 4, 3)      # t, p, kp, tok_r, half
        v = v.reshape(TT, P, NK8 // 2, 2, P)
    return np.ascontiguousarray(v).astype(ml_dtypes.float8_e4m3)


def kernel(**inputs: np.ndarray) -> np.ndarray:
    import ml_dtypes
    x = np.ascontiguousarray(np.asarray(inputs["x"], dtype=np.float32).reshape(NTOK, D))
    T = NTOK // NCORES
    xh = x.astype(np.float16)
    if XL0:
        xl = None
    else:
        xl32 = x - xh.astype(np.float32)
        if XL8:
            xl = np.clip(xl32 * 2.0 ** 15, -240.0, 240.0).astype(
                ml_dtypes.float8_e4m3)
        else:
            xl = xl32.astype(np.float16)
    shards = [(xh[i * T:(i + 1) * T],
               None if XL0 else xl[i * T:(i + 1) * T],
               x[i * T:(i + 1) * T]) for i in range(NCORES)]
    W = np.asarray(inputs["W_base"], dtype=np.float32)
    Wt = (64.0 * W[D16:]).reshape(NK8 // 2, 2, P, D)
    common = {
        "W16": np.ascontiguousarray((64.0 * W[:D16]).astype(np.float16)),
        "W8": np.ascontiguousarray(Wt.transpose(0, 2, 1, 3)).astype(
            ml_dtypes.float8_e4m3),
        "b_base": 64.0 * np.ascontiguousarray(
            inputs["b_base"], dtype=np.float32).reshape(1, D),
        "A_s": np.ascontiguousarray(inputs["A_s"], dtype=np.float32),
        "B_s": 64.0 * np.ascontiguousarray(inputs["B_s"], dtype=np.float32),
        "A_r": np.ascontiguousarray(inputs["A_r"], dtype=np.float32),
        "B_r": 64.0 * np.ascontiguousarray(inputs["B_r"], dtype=np.float32),
        "W_router": np.ascontiguousarray(inputs["W_router"], dtype=np.float32),
        "b_router": np.ascontiguousarray(inputs["b_router"], dtype=np.float32).reshape(1, E),
    }
    if TR1:
        expt = np.zeros((9, 112), dtype=np.float16)
        for e in range(E):
            expt[e, 16 * e:16 * (e + 1)] = 1.0
        expt[8, 96:112] = 1.0
        common["EXPT"] = expt
        onesrow = np.zeros((32, P), dtype=np.float16)
        onesrow[16, :] = 1.0          # partition 96+16 = 112: the bias row
        common["onesrow"] = onesrow
    xlkey = "xl8T" if XL8 else "xlT"
    in_maps = [dict(common,
                    **{"xhT": _pack_xT(sh), "x8T": _pack_x8(sf)},
                    **({} if XL0 else {xlkey: _pack_xT(sl)}))
               for sh, sl, sf in shards]
    nc = _get_nc(T)
    res = bass_utils.run_bass_kernel_spmd(nc, in_maps, core_ids=list(range(NCORES)))
    out = np.concatenate([res.results[i]["y"].astype(np.float32)
                          for i in range(NCORES)], axis=0)
    return out.reshape(B, S, D)


if __name__ == "__main__":
    rng = np.random.default_rng(0)
    demo = {
        "x": rng.standard_normal((B, S, D), dtype=np.float32),
        "W_base": 0.02 * rng.standard_normal((D, D), dtype=np.float32),
        "b_base": 0.02 * rng.standard_normal((D,), dtype=np.float32),
        "A_s": 0.02 * rng.standard_normal((D, R), dtype=np.float32),
        "B_s": 0.02 * rng.standard_normal((R, D), dtype=np.float32),
        "A_r": 0.02 * rng.standard_normal((E, D, R), dtype=np.float32),
        "B_r": 0.02 * rng.standard_normal((E, R, D), dtype=np.float32),
        "W_router": 0.02 * rng.standard_normal((D, E), dtype=np.float32),
        "b_router": 0.02 * rng.standard_normal((E,), dtype=np.float32),
    }
    y = kernel(**demo)
    print("kernel ran, output", y.shape, y.dtype)

